# revision 1
# baseline (speedup 1.0000x reference)
"""Trainium2 Bass kernel for nn_Attention_52536039965434.

Reference computation (B=2, SQ=SK=2048, H=1024, NH=16, HD=64):
    qkv = x @ c_attn_w + b ; per-head attention with multiplicative mask
    (post-score, pre-softmax); attn @ c_proj_w + b; gelu(cat(x, attn) @ mlp_w + b)

Sharding (8 cores): core c -> (b = c//4, g = c%4). Data parallel over batch,
tensor parallel over 4 head-groups (4 heads = 256 dims each). Everything on
device is kept transposed (feature-major: [feat, seq]) so all matmuls contract
over the partition axis. Two 4-rank AllGathers stitch the tensor-parallel
pieces back together (attn^T before c_proj, z^T before the MLP's second half).

Key identities (validated against the jax reference in numpy):
  - mask applied by scaling K columns: S' = Q (K*m)^T == (Q K^T) * m
  - softmax without max-subtraction (scores are O(15) here, exp is safe in f32)
  - denominator = 65th PV output column (V augmented with a ones column)
  - Q/K/V biases via an augmented contraction row (ones row in x^T, bias row
    in W); c_proj bias via ACT per-partition bias; mlp bias via aug row on a
    constant ones vector
All matmuls run as float32r (TF32-like, full PE rate at free-dim >= 256).
"""

import os

import numpy as np

import concourse.bacc as bacc
import concourse.mybir as mybir
import concourse.tile as tile
from concourse import bass_utils

# ---- problem dims (hardcoded per contest contract) ----
B = 2
S = 2048          # SQ == SK
H = 1024
NH = 16
HD = 64
NCORES = 8
TP = 4            # cores per batch (head groups)
HPC = NH // TP    # heads per core = 4
DH = HPC * HD     # per-core head width = 256
QB = 512          # q-block (matmul moving free dim)
P = 128

F32 = mybir.dt.float32
F32R = mybir.dt.float32r
AF = mybir.ActivationFunctionType
ALU = mybir.AluOpType


def _build_nc(s=S, gelu_mode=None, reps=1, ag_mode=None):
    """Build + compile the single SPMD Bass program (same NEFF on all 8 cores)."""
    if gelu_mode is None:
        gelu_mode = os.environ.get("KERNEL_GELU", "builtin")
    if ag_mode is None:
        ag_mode = os.environ.get("KERNEL_AG", "cc")  # cc | dma (timing expt)

    def _allgather(nc_, src_ap, dst_ap, rg_):
        if ag_mode == "cc":
            nc_.gpsimd.collective_compute(
                "AllGather", ALU.bypass, replica_groups=rg_,
                ins=[src_ap.opt()], outs=[dst_ap.opt()],
            )
        else:
            # timing experiment: same bytes moved, no collective (wrong values)
            for r in range(4):
                nc_.sync.dma_start(
                    out=dst_ap[r * DH : (r + 1) * DH, :], in_=src_ap[:]
                )
    nq = s // QB          # q blocks
    nkt = s // P          # k tiles
    nf = H // P           # feature tiles of H = 8

    nc = bacc.Bacc(
        "TRN2", target_bir_lowering=False, debug=False, num_devices=NCORES
    )

    # ---- kernel I/O (per-core contents supplied via in_maps) ----
    xatt = nc.dram_tensor("xatt", [H + 1, s], F32R, kind="ExternalInput").ap()
    xatd = nc.dram_tensor("xatd", [H + 1, s], F32R, kind="ExternalInput").ap()
    wq_d = nc.dram_tensor("wq", [H + 1, DH], F32R, kind="ExternalInput").ap()
    wk_d = nc.dram_tensor("wk", [H + 1, DH], F32R, kind="ExternalInput").ap()
    wv_d = nc.dram_tensor("wv", [H + 1, DH], F32R, kind="ExternalInput").ap()
    mask_d = nc.dram_tensor("maskrep", [P, s], F32, kind="ExternalInput").ap()
    cpw_d = nc.dram_tensor("cprojw", [H, DH], F32R, kind="ExternalInput").ap()
    cpb_d = nc.dram_tensor("cprojb", [P, 2], F32, kind="ExternalInput").ap()
    mw_d = nc.dram_tensor("mlpw", [2 * H, DH], F32R, kind="ExternalInput").ap()
    mb_d = nc.dram_tensor("mlpb", [1, DH], F32R, kind="ExternalInput").ap()
    outT = nc.dram_tensor("outT", [DH, s], F32, kind="ExternalOutput").ap()

    rg = [[0, 1, 2, 3], [4, 5, 6, 7]]

    with tile.TileContext(nc) as tc:
      for rep in range(reps):
        with (
            tc.tile_pool(name=f"xstream{rep}", bufs=4) as xstream,
            tc.tile_pool(name=f"dram{rep}", bufs=1, space="DRAM") as dram,
            tc.tile_pool(name=f"psA{rep}", bufs=2, space="PSUM") as psA,
        ):
            # internal DRAM for collectives (split by q-halves for overlap)
            nhalf = 2 if nq >= 2 else 1
            qpb = nq // nhalf          # q-blocks per half
            HF = s // nhalf
            ag1_in = [dram.tile([DH, HF], F32R, tag=f"ag1in{hf}", name=f"ag1_in{hf}")
                      for hf in range(nhalf)]
            ag1_out = [dram.tile([H, HF], F32R, tag=f"ag1out{hf}", name=f"ag1_out{hf}")
                       for hf in range(nhalf)]
            ag2_in = [dram.tile([DH, HF], F32R, tag=f"ag2in{hf}", name=f"ag2_in{hf}")
                      for hf in range(nhalf)]
            ag2_out = [dram.tile([H, HF], F32R, tag=f"ag2out{hf}", name=f"ag2_out{hf}")
                       for hf in range(nhalf)]

            with (
                tc.tile_pool(name=f"w1{rep}", bufs=1) as w1,
                tc.tile_pool(name=f"qkvp{rep}", bufs=1) as qkvp,
                tc.tile_pool(name=f"attnp{rep}", bufs=1) as attnp,
                tc.tile_pool(name=f"small{rep}", bufs=4) as small,
                tc.tile_pool(name=f"epool{rep}", bufs=8) as epool,
                tc.tile_pool(name=f"augstream{rep}", bufs=2) as augstream,
            ):
                # ---------- weight / mask loads for phase 1 ----------
                wq_sb = w1.tile([P, nf * DH], F32R, tag="wq")
                wk_sb = w1.tile([P, nf * DH], F32R, tag="wk")
                wv_sb = w1.tile([P, nf * DH], F32R, tag="wv")
                wqb_sb = w1.tile([1, DH], F32R, tag="wqb")
                wkb_sb = w1.tile([1, DH], F32R, tag="wkb")
                wvb_sb = w1.tile([1, DH], F32R, tag="wvb")
                mask_sb = w1.tile([P, s], F32, tag="mask")
                for w_d, w_sb, wb_sb in (
                    (wq_d, wq_sb, wqb_sb),
                    (wk_d, wk_sb, wkb_sb),
                    (wv_d, wv_sb, wvb_sb),
                ):
                    nc.sync.dma_start(
                        out=w_sb[:].rearrange("p (t d) -> p t d", d=DH),
                        in_=w_d[:H].rearrange("(t p) d -> p t d", p=P),
                    )
                    nc.sync.dma_start(out=wb_sb[:], in_=w_d[H : H + 1])
                nc.sync.dma_start(out=mask_sb[:], in_=mask_d[:])

                # persistent per-phase activations
                QT_sb = qkvp.tile([P, 2 * s], F32R, tag="qt")   # pair p at cols p*s
                KT_sb = qkvp.tile([P, 2 * s], F32R, tag="kt")
                V_sb = qkvp.tile([P, nkt * 260], F32R, tag="v") # per kt: 4 heads x 65
                attnT_sb = attnp.tile([P, 2 * s], F32R, tag="attnT")

                # ones columns of the augmented V (denominator trick)
                for kt in range(nkt):
                    nc.vector.memset(
                        V_sb[:, kt * 260 : (kt + 1) * 260]
                        .rearrange("p (h c) -> p h c", c=65)[:, :, 64:65]
                        .opt()
                        .bitcast(F32),
                        1.0,
                    )

                # ---------- phase 1a: Q^T ----------
                for kb in range(nq):
                    cs = slice(kb * QB, (kb + 1) * QB)
                    x_ch = [
                        xstream.tile([P, (nf // 2) * QB], F32R, tag="xch",
                                     name=f"xq{kb}_{i}")
                        for i in range(2)
                    ]
                    x_aug = augstream.tile([1, QB], F32R, tag="xaug")
                    for i in range(2):
                        nc.sync.dma_start(
                            out=x_ch[i][:].rearrange("p (t q) -> p t q", q=QB),
                            in_=xatt[i * (H // 2) : (i + 1) * (H // 2)]
                            .rearrange("(t p) q -> p t q", p=P)[:, :, cs],
                        )
                    nc.sync.dma_start(out=x_aug[:], in_=xatt[H : H + 1, cs])
                    for p in range(2):
                        ps = psA.tile([P, QB], F32, tag="acc512")
                        for t in range(nf):
                            nc.tensor.matmul(
                                ps[:],
                                lhsT=w_slice(wq_sb, t, p),
                                rhs=x_ch[t // 4][
                                    :, (t % 4) * QB : (t % 4 + 1) * QB
                                ],
                                start=(t == 0),
                                stop=False,
                            )
                        nc.tensor.matmul(
                            ps[:],
                            lhsT=wqb_sb[0:1, p * P : (p + 1) * P],
                            rhs=x_aug[:],
                            start=False,
                            stop=True,
                        )
                        nc.vector.tensor_copy(
                            QT_sb[:, p * s + kb * QB : p * s + (kb + 1) * QB],
                            ps[:],
                        )

                # ---------- phase 1b: K^T (masked) and V (k-major) ----------
                for kb in range(nq):
                    cs = slice(kb * QB, (kb + 1) * QB)
                    x_ch = [
                        xstream.tile([P, (nf // 2) * QB], F32R, tag="xch",
                                     name=f"xd{kb}_{i}")
                        for i in range(2)
                    ]
                    x_aug = augstream.tile([1, QB], F32R, tag="xaug")
                    for i in range(2):
                        nc.sync.dma_start(
                            out=x_ch[i][:].rearrange("p (t q) -> p t q", q=QB),
                            in_=xatd[i * (H // 2) : (i + 1) * (H // 2)]
                            .rearrange("(t p) q -> p t q", p=P)[:, :, cs],
                        )
                    nc.sync.dma_start(out=x_aug[:], in_=xatd[H : H + 1, cs])
                    for p in range(2):
                        ps = psA.tile([P, QB], F32, tag="acc512")
                        for t in range(nf):
                            nc.tensor.matmul(
                                ps[:],
                                lhsT=w_slice(wk_sb, t, p),
                                rhs=x_ch[t // 4][
                                    :, (t % 4) * QB : (t % 4 + 1) * QB
                                ],
                                start=(t == 0),
                                stop=False,
                            )
                        nc.tensor.matmul(
                            ps[:],
                            lhsT=wkb_sb[0:1, p * P : (p + 1) * P],
                            rhs=x_aug[:],
                            start=False,
                            stop=True,
                        )
                        # fused mask-by-K evacuation
                        nc.vector.tensor_tensor(
                            KT_sb[:, p * s + kb * QB : p * s + (kb + 1) * QB],
                            ps[:],
                            mask_sb[:, cs],
                            ALU.mult,
                        )
                    for sub in range(QB // P):
                        kt = kb * (QB // P) + sub
                        psv = psA.tile([P, DH], F32, tag="acc512")
                        for t in range(nf):
                            nc.tensor.matmul(
                                psv[:],
                                lhsT=x_ch[t // 4][
                                    :,
                                    (t % 4) * QB + sub * P : (t % 4) * QB
                                    + (sub + 1) * P,
                                ],
                                rhs=wv_sb[:, t * DH : (t + 1) * DH],
                                start=(t == 0),
                                stop=False,
                            )
                        nc.tensor.matmul(
                            psv[:],
                            lhsT=x_aug[0:1, sub * P : (sub + 1) * P],
                            rhs=wvb_sb[:],
                            start=False,
                            stop=True,
                        )
                        nc.vector.tensor_copy(
                            V_sb[:, kt * 260 : (kt + 1) * 260]
                            .rearrange("p (h c) -> p h c", c=65)[:, :, 0:64],
                            psv[:].rearrange("p (h c) -> p h c", c=HD),
                        )

                # ---------- phase 2: attention ----------
                for qb in range(nq):
                    for p in range(2):
                        qs = slice(p * s + qb * QB, p * s + (qb + 1) * QB)
                        pvs = [
                            psA.tile([65, QB], F32, tag="pv", name=f"pv{_h}")
                            for _h in range(2)
                        ]
                        for c2 in range(nkt // 2):
                            sstiles = [
                                psA.tile([P, 2 * QB], F32, tag="sc", name=f"sc{_h}")
                                for _h in range(2)
                            ]
                            for j in range(2):
                                kt = 2 * c2 + j
                                for half in range(2):
                                    nc.tensor.matmul(
                                        sstiles[half][:, j * QB : (j + 1) * QB],
                                        lhsT=KT_sb[
                                            64 * half : 64 * half + 64,
                                            p * s + kt * P : p * s + (kt + 1) * P,
                                        ],
                                        rhs=QT_sb[64 * half : 64 * half + 64, qs]
                                        ,
                                        start=True,
                                        stop=True,
                                        tile_position=(64 * half, 0),
                                    )
                            etiles = []
                            for half in range(2):
                                e = epool.tile(
                                    [P, 2 * QB], F32R, tag="e", name=f"e{half}"
                                )
                                nc.scalar.activation(e[:], sstiles[half][:], AF.Exp)
                                etiles.append(e)
                            for j in range(2):
                                kt = 2 * c2 + j
                                for half in range(2):
                                    h = 2 * p + half
                                    nc.tensor.matmul(
                                        pvs[half][:],
                                        lhsT=V_sb[
                                            :,
                                            kt * 260 + h * 65 : kt * 260
                                            + (h + 1) * 65,
                                        ],
                                        rhs=etiles[half][:, j * QB : (j + 1) * QB]
                                        ,
                                        start=(kt == 0),
                                        stop=(kt == nkt - 1),
                                    )
                        # normalize by the denominator (row 64), store attn^T
                        for half in range(2):
                            rec = small.tile([1, QB], F32, tag="rec")
                            nc.vector.reciprocal(rec[:], pvs[half][64:65, :])
                            recb = small.tile([64, QB], F32, tag="recb")
                            nc.gpsimd.partition_broadcast(recb[:], rec[:], channels=64)
                            nc.vector.tensor_tensor(
                                attnT_sb[64 * half : 64 * half + 64, qs],
                                pvs[half][0:64, :],
                                recb[:],
                                ALU.mult,
                            )
                        nc.sync.dma_start(
                            out=ag1_in[qb // qpb][
                                p * P : (p + 1) * P,
                                (qb % qpb) * QB : (qb % qpb + 1) * QB,
                            ],
                            in_=attnT_sb[:, qs],
                        )

            # ================= tail: AG1, mlp1, c_proj, AG2, mlp2 ==========
            with (
                tc.tile_pool(name=f"tailw{rep}", bufs=1) as tailw,
                tc.tile_pool(name=f"big{rep}", bufs=3) as bigpool,
                tc.tile_pool(name=f"out1z{rep}", bufs=1) as out1z,
                tc.tile_pool(name=f"gtmp{rep}", bufs=3) as gtmp,
            ):
                cproj_sb = tailw.tile([P, nf * DH], F32R, tag="cproj")
                cprojb_sb = tailw.tile([P, 2], F32, tag="cprojb")
                mlp_sb = tailw.tile([P, 2 * nf * DH], F32R, tag="mlp")
                mlpb_sb = tailw.tile([1, DH], F32R, tag="mlpb")
                ones_sb = tailw.tile([1, QB], F32R, tag="ones")
                nc.sync.dma_start(
                    out=cproj_sb[:].rearrange("p (t d) -> p t d", d=DH),
                    in_=cpw_d[:].rearrange("(t p) d -> p t d", p=P),
                )
                nc.sync.dma_start(out=cprojb_sb[:], in_=cpb_d[:])
                nc.sync.dma_start(
                    out=mlp_sb[:].rearrange("p (t d) -> p t d", d=DH),
                    in_=mw_d[:].rearrange("(t p) d -> p t d", p=P),
                )
                nc.sync.dma_start(out=mlpb_sb[:], in_=mb_d[:])
                nc.vector.memset(ones_sb[:].bitcast(F32), 1.0)

                out1_sb = out1z.tile([P, 2 * s], F32, tag="out1")
                z_sb = out1z.tile([P, 2 * s], F32R, tag="z")

                # ---- AllGather 1 (attn heads), per q-half ----
                attn_half = []
                for hf in range(nhalf):
                    _allgather(nc, ag1_in[hf][:], ag1_out[hf][:], rg)
                    ah = bigpool.tile([P, nf * HF], F32R, tag="big",
                                      name=f"attnh{hf}")
                    for ht in range(nf):
                        nc.sync.dma_start(
                            out=ah[:, ht * HF : (ht + 1) * HF],
                            in_=ag1_out[hf][ht * P : (ht + 1) * P, :],
                        )
                    attn_half.append(ah)

                # ---- mlp first half (overlaps AG1) ----
                for qb in range(nq):
                    cs = slice(qb * QB, (qb + 1) * QB)
                    x_ch = [
                        xstream.tile([P, (nf // 2) * QB], F32R, tag="xch",
                                     name=f"xm{qb}_{i}")
                        for i in range(2)
                    ]
                    for i in range(2):
                        nc.sync.dma_start(
                            out=x_ch[i][:].rearrange("p (t q) -> p t q", q=QB),
                            in_=xatt[i * (H // 2) : (i + 1) * (H // 2)]
                            .rearrange("(t p) q -> p t q", p=P)[:, :, cs],
                        )
                    for ct in range(2):
                        ps = psA.tile([P, QB], F32, tag="acc512")
                        for t in range(nf):
                            nc.tensor.matmul(
                                ps[:],
                                lhsT=w_slice(mlp_sb, t, ct),
                                rhs=x_ch[t // 4][
                                    :, (t % 4) * QB : (t % 4 + 1) * QB
                                ],
                                start=(t == 0),
                                stop=(t == nf - 1),
                            )
                        nc.vector.tensor_copy(
                            out1_sb[:, ct * s + qb * QB : ct * s + (qb + 1) * QB],
                            ps[:],
                        )

                # ---- c_proj (z^T, o-column shard) + AllGather 2, per q-half ----
                for hf in range(nhalf):
                    for ot in range(2):
                        for qb in range(hf * qpb, (hf + 1) * qpb):
                            qo = (qb % qpb) * QB
                            ps = psA.tile([P, QB], F32, tag="acc512")
                            for ht in range(nf):
                                nc.tensor.matmul(
                                    ps[:],
                                    lhsT=w_slice(cproj_sb, ht, ot),
                                    rhs=attn_half[hf][
                                        :, ht * HF + qo : ht * HF + qo + QB
                                    ],
                                    start=(ht == 0),
                                    stop=(ht == nf - 1),
                                )
                            nc.scalar.activation(
                                z_sb[:, ot * s + qb * QB : ot * s + (qb + 1) * QB],
                                ps[:],
                                AF.Identity,
                                bias=cprojb_sb[:, ot : ot + 1],
                            )
                        nc.sync.dma_start(
                            out=ag2_in[hf][ot * P : (ot + 1) * P, :],
                            in_=z_sb[:, ot * s + hf * HF : ot * s + (hf + 1) * HF],
                        )
                    _allgather(nc, ag2_in[hf][:], ag2_out[hf][:], rg)
                # ---- mlp second half + gelu (z streamed per q-block) ----
                for qb in range(nq):
                    hf = qb // qpb
                    qo = (qb % qpb) * QB
                    zch = bigpool.tile([P, nf * QB], F32R, tag="big",
                                       name=f"zch{qb}")
                    for ot8 in range(nf):
                        nc.sync.dma_start(
                            out=zch[:, ot8 * QB : (ot8 + 1) * QB],
                            in_=ag2_out[hf][ot8 * P : (ot8 + 1) * P, qo : qo + QB],
                        )
                    for ct in range(2):
                        ps = psA.tile([P, QB], F32, tag="acc512")
                        for ot8 in range(nf):
                            nc.tensor.matmul(
                                ps[:],
                                lhsT=w_slice(mlp_sb, nf + ot8, ct),
                                rhs=zch[:, ot8 * QB : (ot8 + 1) * QB],
                                start=(ot8 == 0),
                                stop=False,
                            )
                        # + mlp_b via aug row on constant ones
                        nc.tensor.matmul(
                            ps[:],
                            lhsT=mlpb_sb[0:1, ct * P : (ct + 1) * P],
                            rhs=ones_sb[0:1, :],
                            start=False,
                            stop=True,
                        )
                        tmp = gtmp.tile([P, QB], F32, tag="gtmp")
                        nc.vector.tensor_add(
                            tmp[:],
                            ps[:],
                            out1_sb[:, ct * s + qb * QB : ct * s + (qb + 1) * QB],
                        )
                        gout = gtmp.tile([P, QB], F32, tag="gout")
                        if gelu_mode == "builtin":
                            nc.scalar.activation(gout[:], tmp[:], AF.Gelu_apprx_tanh)
                        else:
                            # exact GPT-2 tanh gelu from primitives
                            u = gtmp.tile([P, QB], F32, tag="gu")
                            th = gtmp.tile([P, QB], F32, tag="gth")
                            nc.vector.tensor_mul(u[:], tmp[:], tmp[:])      # x^2
                            nc.vector.tensor_mul(u[:], u[:], tmp[:])        # x^3
                            nc.vector.scalar_tensor_tensor(
                                u[:], u[:], 0.044715, tmp[:], ALU.mult, ALU.add
                            )                                               # x + a x^3
                            nc.scalar.activation(
                                th[:], u[:], AF.Tanh,
                                scale=0.7978845608028654,
                            )
                            nc.vector.scalar_tensor_tensor(
                                th[:], th[:], 1.0, tmp[:], ALU.add, ALU.mult
                            )                                               # (1+t)*x
                            nc.vector.tensor_scalar_mul(gout[:], th[:], 0.5)
                        nc.sync.dma_start(
                            out=outT[
                                ct * P : (ct + 1) * P, qb * QB : (qb + 1) * QB
                            ],
                            in_=gout[:],
                        )

    nc.compile()
    return nc


def w_slice(w_sb, t, p):
    """lhsT [128, 128] slice: f-tile t, output half p, of a [128, nt*256] layout."""
    return w_sb[:, t * DH + p * P : t * DH + (p + 1) * P]


def x_ch_slice(x_ch, t, sub):
    """lhsT tile [128 f, 128 k] out of a streamed x^T chunk [128, nf*QB]."""
    return x_ch[:, t * QB + sub * P : t * QB + (sub + 1) * P]


_NC_CACHE = {}
LAST_RESULTS = None


def _get_nc(s=S):
    if s not in _NC_CACHE:
        _NC_CACHE[s] = _build_nc(s)
    return _NC_CACHE[s]


def _get_nc_reps(reps):
    key = ("reps", reps)
    if key not in _NC_CACHE:
        _NC_CACHE[key] = _build_nc(reps=reps)
    return _NC_CACHE[key]


def kernel(**inputs):
    global LAST_RESULTS
    nc = _get_nc()
    in_maps = make_in_maps(inputs)

    trace = bool(int(os.environ.get("KERNEL_TRACE", "0")))
    res = bass_utils.run_bass_kernel_spmd(
        nc, in_maps, core_ids=list(range(NCORES)), trace=trace
    )
    LAST_RESULTS = res

    out = np.empty((B, S, H), np.float32)
    for c in range(NCORES):
        b, g = c // TP, c % TP
        out[b, :, g * DH : (g + 1) * DH] = res.results[c]["outT"].T
    return out


def make_in_maps(inputs):
    xq = np.ascontiguousarray(np.asarray(inputs["attender_seq"], np.float32))
    xk = np.ascontiguousarray(np.asarray(inputs["attendee_seq"], np.float32))
    mask = np.asarray(inputs["attendee_mask"]).astype(np.float32)
    caw = np.asarray(inputs["c_attn_w"], np.float32)
    cab = np.asarray(inputs["c_attn_b"], np.float32)
    cpw = np.ascontiguousarray(np.asarray(inputs["c_proj_w"], np.float32))
    cpb = np.asarray(inputs["c_proj_b"], np.float32)
    mw = np.ascontiguousarray(np.asarray(inputs["mlp_w"], np.float32))
    mb = np.asarray(inputs["mlp_b"], np.float32)

    in_maps = []
    for c in range(NCORES):
        b, g = c // TP, c % TP
        gs = slice(g * DH, (g + 1) * DH)
        xattT = np.concatenate([xq[b].T, np.ones((1, S), np.float32)], 0)
        xatdT = np.concatenate([xk[b].T, np.ones((1, S), np.float32)], 0)
        wq = np.concatenate([caw[:, gs], cab[None, gs]], 0)
        wk = np.concatenate(
            [caw[:, H + g * DH : H + (g + 1) * DH],
             cab[None, H + g * DH : H + (g + 1) * DH]], 0)
        wv = np.concatenate(
            [caw[:, 2 * H + g * DH : 2 * H + (g + 1) * DH],
             cab[None, 2 * H + g * DH : 2 * H + (g + 1) * DH]], 0)
        in_maps.append({
            "xatt": np.ascontiguousarray(xattT),
            "xatd": np.ascontiguousarray(xatdT),
            "wq": np.ascontiguousarray(wq),
            "wk": np.ascontiguousarray(wk),
            "wv": np.ascontiguousarray(wv),
            "maskrep": np.ascontiguousarray(
                np.broadcast_to(mask[b][None, :], (P, S))),
            "cprojw": np.ascontiguousarray(cpw[:, gs]),
            "cprojb": np.ascontiguousarray(cpb[gs].reshape(2, P).T),
            "mlpw": np.ascontiguousarray(mw[:, gs]),
            "mlpb": np.ascontiguousarray(mb[None, gs]),
        })
    return in_maps

